# revision 14
# baseline (speedup 1.0000x reference)
"""Trainium2 Bass kernel for nn_EnsembleFormer (vq_codebook).

Strategy
--------
Every projected stream (p, k0, v0, k1, v1) in the reference is consumed only
AFTER spatial pooling (64x64 -> 8x8 agents, and 8x8 -> 2x2 clusters), and the
final output is a bilinear upsample of an 8x8 map followed by a 1x1 conv.
Pointwise (1x1) projections commute with average pooling and with bilinear
resize, so the whole network collapses to:

  pool x (64x64 -> 8x8)  ->  tiny GEMMs + clustering on the 8x8 grid
  ->  1x1 proj2 on the 8x8 grid  ->  bilinear upsample 8->64 (a matmul)

which turns a ~29 GFLOP problem into a memory-bound one (read x, write out).

Sharding: data-parallel over batch B=16 across 8 cores (2 batches/core),
weights replicated. No collectives.

Performance structure:
 - DMA-bound: reads 12.6MB of x + ~1.8MB consts; output written in fp16
   (adds ~3e-4 rel err, well under tolerance) halving write traffic; the
   host upcasts to f32.
 - all input DMA triggers are issued up front on the sync queue; output DMAs
   are triggered from the Activation engine so they never block reads.
 - per-batch pipelines overlap: batch 0's chain/upsample/writes run while
   batch 1's input streams in. Batch 1's cluster-chain stages are emitted
   interleaved between batch 0's upsample groups so no engine queue holds
   batch 1's critical path behind batch 0's bulk work.
 - upsample PSUM tiles are [128,1024] (2 matmuls per PSUM->SBUF cast) and the
   walrus LDWEIGHTS dedup is enabled (8 consecutive matmuls share lhsT).

Layout notes (hardware legality):
 - heads are padded 24 -> 32 partitions so all per-head matmul operands sit at
   32-aligned partition bases;
 - PE-transpose outputs must start at PSUM partition 0, so all transposes are
   emitted at base 0 and consumers slice 32-aligned blocks;
 - b1's padded bias rows carry 1.0 for k/v streams, which simultaneously gives
   the distance-constant fold and free per-cluster counts after transposition.
"""

import sys
import numpy as np

try:
    import concourse.bass as bass
except ImportError:  # pragma: no cover
    sys.path.insert(0, "/opt/trn_rl_repo")
    import concourse.bass as bass

from contextlib import ExitStack

import concourse.tile as tile
import concourse.mybir as mybir
import concourse.bass_utils as bass_utils
from concourse.bass_utils import run_bass_kernel_spmd

f32 = mybir.dt.float32
f32r = mybir.dt.float32r
f16 = mybir.dt.float16
AX = mybir.AxisListType
ALU = mybir.AluOpType
ACTF = mybir.ActivationFunctionType

# problem dims
B, CIN, H, W = 16, 384, 64, 64
HEADS, HD, HP = 4, 24, 32          # head dim padded 24 -> 32
NSTREAM = 5
AG, CL = 8, 2
NPIX = AG * AG                      # 64 agent pixels
NCORES = 8
BPC = B // NCORES                   # batches per core = 2

# packed const layout (columns of the [128, C_END] f32 const tensor)
C_W1, C_W2, C_B1, C_I, C_E1, C_E1H, C_AL, C_BB = (
    0, 1920, 2304, 2309, 2437, 2438, 2442, 2444)
C_B2 = 2476
C_END = 2479

_CACHE = {}


def _view(t, dims, offset_elems=0):
    """Strided free-dim view of a 2D tile: dims = [[step, count], ...]."""
    return bass.AP(tensor=t.tensor, offset=t.offset + offset_elems,
                   ap=[list(t.ap[0])] + [list(d) for d in dims])


def _upsample_R():
    # jax.image.resize(method='bilinear') 8 -> 64: triangle kernel, half-pixel
    # centers, weights normalized per output row. Verified exact vs jax.
    o = np.arange(64)
    t = (o + 0.5) * (8.0 / 64.0) - 0.5
    i = np.arange(8)
    w = np.maximum(0.0, 1.0 - np.abs(t[:, None] - i[None, :]))
    w = w / w.sum(axis=1, keepdims=True)
    return w.astype(np.float32)


def _split_multi_waits(nc):
    """This container's walrus rejects >1 semaphore wait per instruction;
    move extra waits onto same-engine no-ops inserted just before."""
    n = 0
    for fn in nc.m.functions:
        for bb in fn.blocks:
            new_list = []
            for inst in bb.instructions:
                si = inst.sync_info
                if si is not None and si.on_wait and len(si.on_wait) > 1:
                    waits = list(si.on_wait)
                    for wt in waits[:-1]:
                        nop = mybir.InstNoOp(
                            name=f"waitsplit-{n}", engine=inst.engine,
                            ins=[], outs=[],
                            sync_info=mybir.SyncInfo(on_wait=[wt], on_update=[]),
                        )
                        n += 1
                        new_list.append(nop)
                    si.on_wait = [waits[-1]]
                new_list.append(inst)
            bb.instructions = new_list
    return n


def _build_nc():
    nc = bass.Bass("TRN2")
    xc = nc.dram_tensor("xc", (BPC, CIN, H, W), f32, kind="ExternalInput")
    cst = nc.dram_tensor("cst", (128, C_END), f32, kind="ExternalInput")
    upw2 = nc.dram_tensor("upw2", (64, 4096), f16, kind="ExternalInput")
    outc = nc.dram_tensor("outc", (BPC, CIN, H, W), f16, kind="ExternalOutput")

    def mm(out, lhsT, rhs, start=True, stop=True, tp=(0, 0)):
        nc.tensor.matmul(
            out, lhsT=lhsT, rhs=rhs, start=start, stop=stop,
            tile_position=tp,
        )

    with tile.TileContext(nc) as tc, ExitStack() as ctx:
        const = ctx.enter_context(tc.tile_pool(name="const", bufs=1))
        xin = ctx.enter_context(tc.tile_pool(name="xin", bufs=1))
        mid = ctx.enter_context(tc.tile_pool(name="mid", bufs=2))
        outp = ctx.enter_context(tc.tile_pool(name="outp", bufs=1))
        ps = ctx.enter_context(tc.tile_pool(name="ps", bufs=4, space="PSUM"))
        psU = ctx.enter_context(tc.tile_pool(name="psU", bufs=2, space="PSUM"))

        # ---- first x chunk before consts so pooling starts ASAP ----
        xts = {}
        xt = xin.tile([128, 4096], f32, name="xt00", tag="x00")
        nc.sync.dma_start(
            out=xt, in_=xc[0, 0:128].rearrange("c h w -> c (h w)"))
        xts[(0, 0)] = xt
        tC = const.tile([128, C_END], f32)
        nc.sync.dma_start(out=tC, in_=cst[:, :])
        tONES = const.tile([1, 64], f32)
        nc.vector.memset(tONES, 1.0)

        tW1 = tC[:, C_W1:C_W2]
        tB1 = tC[:, C_B1:C_I]
        tI = tC[:, C_I:C_E1]
        tE1 = tC[:, C_E1:C_E1H]
        tAL = tC[0:4, C_AL:C_BB]
        tBB = tC[0:64, C_BB:C_B2]
        tB2 = tC[:, C_B2:C_END]
        # f32r matmul operands must come from a rounding compute op, not DMA
        tW2r = const.tile([128, 384], f32r)
        nc.vector.tensor_copy(tW2r, tC[:, C_W2:C_B1])
        tE1Hr = const.tile([128, 4], f32r)
        nc.vector.tensor_copy(tE1Hr, tC[:, C_E1H:C_AL])

        # ---- remaining input DMA triggers up front (sync queue) ----
        for j in range(1, 3):
            xt = xin.tile([128, 4096], f32, name=f"xt0{j}", tag=f"x0{j}")
            nc.sync.dma_start(
                out=xt,
                in_=xc[0, j * 128:(j + 1) * 128].rearrange("c h w -> c (h w)"))
            xts[(0, j)] = xt
        tUPW = const.tile([64, 4096], f16)
        nc.sync.dma_start(out=tUPW, in_=upw2[:, :])
        for j in range(3):
            xt = xin.tile([128, 4096], f32, name=f"xt1{j}", tag=f"x1{j}")
            nc.sync.dma_start(
                out=xt,
                in_=xc[1, j * 128:(j + 1) * 128].rearrange("c h w -> c (h w)"))
            xts[(1, j)] = xt

        def pool_only(b):
            # two-stage 64x64 -> 8x8 sum pool (x 4096 over the two stages):
            # stage 1 reduces the contiguous innermost w-groups, stage 2 the
            # strided h-groups
            xp = mid.tile([128, 192], f32, name=f"xp{b}")
            for j in range(3):
                s1t = mid.tile([128, 512], f32, name=f"s1t{b}{j}", tag="s1")
                nc.vector.tensor_reduce(
                    out=s1t,
                    in_=xts[(b, j)].rearrange("p (g wi) -> p g wi", wi=8),
                    axis=AX.X, op=ALU.add)
                nc.vector.tensor_reduce(
                    out=xp[:, j * 64:(j + 1) * 64],
                    in_=s1t.rearrange("p (hb hi wb) -> p hb wb hi",
                                      hb=8, hi=8, wb=8),
                    axis=AX.X, op=ALU.add)
            return xp

        def project(b, xp):
            py = ps.tile([128, 320], f32, name=f"py{b}", tag="ps")
            for s in range(5):
                for j in range(3):
                    mm(py[:, s * 64:(s + 1) * 64],
                       lhsT=tW1[:, j * 640 + s * 128: j * 640 + (s + 1) * 128],
                       rhs=xp[:, j * 64:(j + 1) * 64],
                       start=(j == 0), stop=(j == 2))
            return py

        def stream_bias(b, py, on_vector):
            S = mid.tile([128, 320], f32, name=f"S{b}")
            for s in range(5):
                if on_vector:
                    bcol = tB1[:, s:s + 1]
                    bview = bass.AP(tensor=bcol.tensor, offset=bcol.offset,
                                    ap=[list(bcol.ap[0])] + [[0, 64]])
                    nc.vector.scalar_tensor_tensor(
                        out=S[:, s * 64:(s + 1) * 64],
                        in0=py[:, s * 64:(s + 1) * 64], scalar=1.0 / 64.0,
                        in1=bview, op0=ALU.mult, op1=ALU.add)
                else:
                    nc.scalar.activation(
                        out=S[:, s * 64:(s + 1) * 64],
                        in_=py[:, s * 64:(s + 1) * 64],
                        func=ACTF.Identity, bias=tB1[:, s:s + 1],
                        scale=1.0 / 64.0)
            return S

        def cluster_chain(b, S, out):
            """Clustering + assignment + proj2 on the pooled 8x8 grid.
            Generator: yields at stage boundaries so the caller can
            interleave other engines' bulk work. Appends the fp16 [64, 384]
            upsample lhsT to `out`."""
            # ---- cluster pools (sum of 16): KC cols (si, m), si=k0,v0,k1,v1
            KC = mid.tile([128, 16], f32, name=f"KC{b}")
            for si in range(4):
                sc = (si + 1) * 64
                nc.vector.tensor_reduce(
                    out=KC[:, si * 4:(si + 1) * 4],
                    in_=S[:, sc:sc + 64].rearrange(
                        "p (mi ii mj jj) -> p mi mj ii jj", mi=2, ii=4, mj=2, jj=4),
                    axis=AX.XY, op=ALU.add)
            yield

            # ---- block-diagonal kc (per mod): KCBD[h*32+c, h*4+m] ----
            KCBD = [None, None]
            KC2BD = [None, None]
            for j in range(2):
                si = 2 * j
                kcbd = mid.tile([128, 16], f32, name=f"kcbd{b}{j}")
                nc.vector.memset(kcbd, 0.0)
                for h in range(4):
                    nc.vector.tensor_copy(
                        kcbd[h * 32:(h + 1) * 32, h * 4:(h + 1) * 4],
                        KC[h * 32:(h + 1) * 32, si * 4:(si + 1) * 4])
                kc2bd = mid.tile([128, 16], f32, name=f"kc2bd{b}{j}")
                nc.vector.tensor_mul(kc2bd, kcbd, kcbd)
                KCBD[j] = kcbd
                KC2BD[j] = kc2bd
            yield

            # ---- cluster sq-norms row [1, 32] cols (j, h, m); E1 = 1/256 mask
            pn2 = ps.tile([1, 32], f32, name=f"pn2{b}", tag="ps")
            for j in range(2):
                mm(pn2[0:1, j * 16:(j + 1) * 16], lhsT=tE1, rhs=KC2BD[j])
            n2row = mid.tile([1, 32], f32, name=f"n2row{b}")
            nc.vector.tensor_copy(n2row, pn2)
            pn2b = ps.tile([64, 32], f32, name=f"pn2b{b}", tag="ps")
            mm(pn2b, lhsT=tONES, rhs=n2row)
            N2B = mid.tile([64, 32], f32, name=f"N2B{b}")
            nc.vector.tensor_copy(N2B, pn2b)
            yield

            # ---- simT[n, (j,h,m)] = kc.k (x16, +16) via block-diag rhs ----
            SIM = mid.tile([64, 32], f32, name=f"SIM{b}")
            for j in range(2):
                s = 1 + 2 * j
                psimt = ps.tile([64, 16], f32, name=f"psimt{b}{j}", tag="ps")
                mm(psimt, lhsT=S[:, s * 64:(s + 1) * 64], rhs=KCBD[j])
                # sim = 0.125*cross_raw - ||kc||^2 (= 2 kc.k - ||kc||^2 + const)
                nc.vector.scalar_tensor_tensor(
                    out=SIM[:, j * 16:(j + 1) * 16], in0=psimt, scalar=0.125,
                    in1=N2B[:, j * 16:(j + 1) * 16],
                    op0=ALU.mult, op1=ALU.subtract)
            yield

            # ---- assignment weights WT[n, (j,h,m)] ----
            WT = mid.tile([64, 32], f32r, name=f"WT{b}")
            HMAX = mid.tile([64, 4], f32, name=f"HMAX{b}")
            nc.vector.tensor_reduce(
                out=HMAX, in_=SIM[:, 0:16].rearrange("p (h m) -> p h m", h=4),
                axis=AX.X, op=ALU.max)
            nc.vector.tensor_tensor(
                out=WT[:, 0:16].rearrange("p (h m) -> p h m", h=4),
                in0=SIM[:, 0:16].rearrange("p (h m) -> p h m", h=4),
                in1=_view(HMAX, [[1, 4], [0, 4]]), op=ALU.is_ge)
            ESOFT = mid.tile([64, 16], f32, name=f"ESOFT{b}")
            nc.scalar.activation(out=ESOFT, in_=SIM[:, 16:32], func=ACTF.Exp)
            SSUM = mid.tile([64, 4], f32, name=f"SSUM{b}")
            nc.vector.tensor_reduce(
                out=SSUM, in_=ESOFT.rearrange("p (h m) -> p h m", h=4),
                axis=AX.X, op=ALU.add)
            SRCP = mid.tile([64, 4], f32, name=f"SRCP{b}")
            nc.vector.reciprocal(SRCP, SSUM)
            nc.vector.tensor_tensor(
                out=WT[:, 16:32].rearrange("p (h m) -> p h m", h=4),
                in0=ESOFT.rearrange("p (h m) -> p h m", h=4),
                in1=_view(SRCP, [[1, 4], [0, 4]]), op=ALU.mult)
            yield

            # ---- per-mod agg in [4(m), 128(h,c pad)] layout ----
            PAGGNT = ps.tile([128, 8], f32, name=f"PAGGNT{b}", tag="ps")
            PAGGT = ps.tile([128, 8], f32, name=f"PAGGT{b}", tag="ps")
            for j in range(2):
                s = 2 + 2 * j
                si = 1 + 2 * j
                pstv = ps.tile([64, 128], f32, name=f"pstv{b}{j}", tag="ps")
                nc.tensor.transpose(out=pstv, in_=S[:, s * 64:(s + 1) * 64],
                                    identity=tI)
                stv = mid.tile([64, 128], f32r, name=f"stv{b}{j}")
                nc.vector.tensor_copy(stv, pstv)

                pagg = ps.tile([4, 104], f32, name=f"pagg{b}{j}", tag="ps")
                for h in range(4):
                    mm(pagg[0:4, h * 26:(h + 1) * 26],
                       lhsT=WT[0:64, j * 16 + h * 4: j * 16 + (h + 1) * 4],
                       rhs=stv[0:64, h * 32: h * 32 + 26])
                pvct = ps.tile([4, 128], f32, name=f"pvct{b}{j}", tag="ps")
                nc.tensor.transpose(out=pvct, in_=KC[:, si * 4:(si + 1) * 4],
                                    identity=tI)
                vcts = mid.tile([4, 128], f32, name=f"vcts{b}{j}")
                nc.vector.tensor_copy(vcts, pvct)

                rc = mid.tile([4, 4], f32, name=f"rc{b}{j}")
                nc.vector.tensor_scalar_add(rc, _view(pagg, [[26, 4]], 24), 1.0)
                nc.vector.reciprocal(rc, rc)
                agg = mid.tile([4, 128], f32, name=f"agg{b}{j}")
                nc.vector.memset(agg, 0.0)
                # agg = (vc/16 + sum_n w v) * 1/(1+count)
                nc.vector.scalar_tensor_tensor(
                    out=_view(agg, [[32, 4], [1, 24]]),
                    in0=_view(vcts, [[32, 4], [1, 24]]), scalar=1.0 / 16.0,
                    in1=_view(pagg, [[26, 4], [1, 24]]),
                    op0=ALU.mult, op1=ALU.add)
                nc.vector.tensor_tensor(
                    out=_view(agg, [[32, 4], [1, 24]]),
                    in0=_view(agg, [[32, 4], [1, 24]]),
                    in1=_view(rc, [[1, 4], [0, 24]]), op=ALU.mult)

                # normalized + alpha-scaled aggn
                sq = mid.tile([4, 128], f32, name=f"sq{b}{j}")
                nc.vector.tensor_mul(sq, agg, agg)
                ns = mid.tile([4, 4], f32, name=f"ns{b}{j}")
                nc.vector.tensor_reduce(
                    out=ns, in_=_view(sq, [[32, 4], [1, 24]]),
                    axis=AX.X, op=ALU.add)
                lnv = mid.tile([4, 4], f32, name=f"lnv{b}{j}")
                nc.scalar.activation(lnv, ns, func=ACTF.Ln)
                sd = mid.tile([4, 4], f32, name=f"sd{b}{j}")
                nc.scalar.activation(sd, lnv, func=ACTF.Exp, scale=0.5)  # sqrt
                nc.vector.tensor_scalar_add(sd, sd, 1e-6)
                rn = mid.tile([4, 4], f32, name=f"rn{b}{j}")
                nc.vector.reciprocal(rn, sd)
                aggn = mid.tile([4, 128], f32, name=f"aggn{b}{j}")
                nc.vector.memset(aggn, 0.0)
                nc.vector.scalar_tensor_tensor(
                    out=_view(aggn, [[32, 4], [1, 24]]),
                    in0=_view(agg, [[32, 4], [1, 24]]), scalar=tAL[:, j:j + 1],
                    in1=_view(rn, [[1, 4], [0, 24]]),
                    op0=ALU.mult, op1=ALU.mult)

                nc.tensor.transpose(out=PAGGNT[:, j * 4:(j + 1) * 4], in_=aggn,
                                    identity=tI[0:4, 0:4])
                nc.tensor.transpose(out=PAGGT[:, j * 4:(j + 1) * 4], in_=agg,
                                    identity=tI[0:4, 0:4])
                yield

            # block-diagonal agg / aggn: [128(h*32+c), 32(h*8+j*4+m)]
            AGGNBD = mid.tile([128, 32], f32r, name=f"AGGNBD{b}")
            nc.vector.memset(AGGNBD.bitcast(f32), 0.0)
            AGGBD = mid.tile([128, 32], f32r, name=f"AGGBD{b}")
            nc.vector.memset(AGGBD.bitcast(f32), 0.0)
            for h in range(4):
                nc.vector.tensor_copy(
                    AGGNBD[h * 32:(h + 1) * 32, h * 8:(h + 1) * 8],
                    PAGGNT[h * 32:(h + 1) * 32, 0:8])
                nc.vector.tensor_copy(
                    AGGBD[h * 32:(h + 1) * 32, h * 8:(h + 1) * 8],
                    PAGGT[h * 32:(h + 1) * 32, 0:8])
            yield

            # ---- pa reciprocal norms [64, 4] per head ----
            s0sq = mid.tile([128, 64], f32r, name=f"s0sq{b}")
            nc.vector.tensor_mul(s0sq, S[:, 0:64], S[:, 0:64])
            ppn2 = ps.tile([64, 4], f32, name=f"ppn2{b}", tag="ps")
            mm(ppn2, lhsT=s0sq, rhs=tE1Hr)
            pad_ = mid.tile([64, 4], f32, name=f"pad_{b}")
            nc.scalar.activation(pad_, ppn2, func=ACTF.Ln, scale=256.0)
            par = mid.tile([64, 4], f32, name=f"par{b}")
            nc.scalar.activation(par, pad_, func=ACTF.Exp, scale=0.5)  # ||pa||
            nc.vector.tensor_scalar_add(par, par, 1e-6)
            nc.vector.reciprocal(par, par)
            yield

            # ---- cosine sim + assignment softmax over all 8 clusters ----
            s0r = mid.tile([128, 64], f32r, name=f"s0r{b}")
            nc.vector.tensor_copy(s0r, S[:, 0:64])
            psimcos = ps.tile([64, 32], f32, name=f"psimcos{b}", tag="ps")
            mm(psimcos, lhsT=s0r, rhs=AGGNBD)
            SC = mid.tile([64, 32], f32, name=f"SC{b}")
            nc.vector.tensor_tensor(
                out=SC, in0=psimcos, in1=_view(par, [[1, 4], [0, 8]]),
                op=ALU.mult)
            nc.vector.tensor_add(SC, SC, tBB)
            EA = mid.tile([64, 32], f32, name=f"EA{b}")
            nc.scalar.activation(EA, SC, func=ACTF.Exp)
            ASUM = mid.tile([64, 4], f32, name=f"ASUM{b}")
            nc.vector.tensor_reduce(
                out=ASUM, in_=EA.rearrange("p (h m) -> p h m", h=4),
                axis=AX.X, op=ALU.add)
            ARCP = mid.tile([64, 4], f32, name=f"ARCP{b}")
            nc.vector.reciprocal(ARCP, ASUM)
            ASSC = mid.tile([64, 32], f32, name=f"ASSC{b}")   # assign (h, j, m)
            nc.vector.tensor_tensor(
                out=ASSC, in0=EA,
                in1=_view(ARCP, [[1, 4], [0, 8]]), op=ALU.mult)
            past = ps.tile([32, 64], f32, name=f"past{b}", tag="ps")
            nc.tensor.transpose(out=past, in_=ASSC, identity=tI[0:64, 0:64])
            asts = mid.tile([32, 64], f32r, name=f"asts{b}")
            nc.vector.tensor_copy(asts, past)
            yield

            # ---- G = agg @ W2: [32 (h,j,m), 384], then q8T = asts.T @ G ----
            pg32 = ps.tile([32, 384], f32, name=f"pg32{b}", tag="ps")
            mm(pg32, lhsT=AGGBD, rhs=tW2r)
            gs = mid.tile([32, 384], f32r, name=f"gs{b}")
            nc.scalar.copy(gs, pg32)
            yield

            pq8t = ps.tile([64, 384], f32, name=f"pq8t{b}", tag="ps")
            mm(pq8t, lhsT=asts, rhs=gs)
            q8f = mid.tile([64, 384], f16, name=f"q8f{b}")
            nc.scalar.copy(q8f, pq8t)
            out.append(q8f)

        def upsample_groups(b, q8f, split):
            """Bilinear upsample 8x8 -> 64x64 via fp16 matmul; +b2 folded into
            the PSUM->SBUF cast; fp16 out. Generator: yields after each group
            (2 matmuls + 1 cast); output DMA triggered from the Activation
            engine after each channel block completes."""
            for jo in range(3):
                osb = outp.tile([128, 4096], f16, name=f"osb{b}{jo}",
                                tag=f"o{jo}")
                for g in range(4):
                    pup = psU.tile([128, 1024], f32, name=f"pup{b}{jo}{g}",
                                   tag="psU")
                    for hh in range(2):
                        nn = g * 2 + hh
                        mm(pup[:, hh * 512:(hh + 1) * 512],
                           lhsT=q8f[0:64, jo * 128:(jo + 1) * 128],
                           rhs=tUPW[0:64, nn * 512:(nn + 1) * 512])
                    dst = osb[:, g * 1024:(g + 1) * 1024]
                    if split and g % 2 == 1:
                        nc.vector.tensor_scalar_add(dst, pup, tB2[:, jo:jo + 1])
                    else:
                        nc.scalar.activation(
                            out=dst, in_=pup, func=ACTF.Identity,
                            bias=tB2[:, jo:jo + 1], scale=1.0)
                    yield
                oj = outc[b, jo * 128:(jo + 1) * 128].rearrange("c h w -> c (h w)")
                nc.scalar.dma_start(out=oj, in_=osb)

        def drain(gen):
            for _ in gen:
                pass

        # ================= schedule =================
        # batch 0 front half: pools (vector), proj (PE), biases (scalar),
        # full chain (all engines free at this point)
        xp0 = pool_only(0)
        py0 = project(0, xp0)
        S0 = stream_bias(0, py0, on_vector=False)
        q8box0 = []
        drain(cluster_chain(0, S0, q8box0))
        q8f0 = q8box0[0]

        # batch 1 pools emitted now: vector runs them as chunks land, while
        # batch 0's upsample occupies PE/scalar
        xp1 = pool_only(1)

        # batch 0 upsample: first half of the groups
        ups0 = upsample_groups(0, q8f0, split=False)
        for _ in range(6):
            next(ups0)

        # batch 1 projection + biases: PE visit between upsample groups
        py1 = project(1, xp1)
        S1 = stream_bias(1, py1, on_vector=True)

        # interleave the rest of batch 0's upsample with batch 1's chain
        # stages so neither engine queue blocks the other batch
        q8box1 = []
        chain1 = cluster_chain(1, S1, q8box1)
        chain_alive = True
        while chain_alive:
            try:
                next(chain1)
            except StopIteration:
                chain_alive = False
            try:
                next(ups0)
            except StopIteration:
                pass
        drain(ups0)
        q8f1 = q8box1[0]

        drain(upsample_groups(1, q8f1, split=True))

    _split_multi_waits(nc)
    return nc


def _host_prep(W1, b1, W2, b2, sim_alpha, sim_beta):
    W1 = np.asarray(W1, np.float32)
    b1 = np.asarray(b1, np.float32)
    W2 = np.asarray(W2, np.float32)
    b2 = np.asarray(b2, np.float32)
    sim_alpha = np.asarray(sim_alpha, np.float32)
    sim_beta = np.asarray(sim_beta, np.float32)

    # W1 padded: [3, 128, 5 streams, 4 heads, 32] -> (128, 1920)
    w1r = W1.reshape(3, 128, NSTREAM, HEADS, HD)
    w1p = np.zeros((3, 128, NSTREAM, HEADS, HP), np.float32)
    w1p[..., :HD] = w1r
    w1p = w1p.reshape(3, 128, NSTREAM * 128).transpose(1, 0, 2).reshape(128, 1920)

    # b1 padded: (128, 5); pad row 24 carries 1.0 for k/v streams
    b1r = b1.reshape(NSTREAM, HEADS, HD)
    b1pad = np.zeros((NSTREAM, HEADS, HP), np.float32)
    b1pad[..., :HD] = b1r
    for s in range(1, NSTREAM):
        b1pad[s, :, HD] = 1.0
    b1p = b1pad.transpose(1, 2, 0).reshape(128, NSTREAM)

    # W2 padded rows 24->32: (128, 384)
    w2r = W2.reshape(HEADS, HD, CIN)
    w2p = np.zeros((HEADS, HP, CIN), np.float32)
    w2p[:, :HD] = w2r
    w2p = w2p.reshape(128, CIN)

    cstm = np.zeros((128, C_END), np.float32)
    cstm[:, C_W1:C_W2] = w1p
    cstm[:, C_W2:C_B1] = w2p
    cstm[:, C_B1:C_I] = b1p
    cstm[:, C_I:C_E1] = np.eye(128, dtype=np.float32)
    cstm[:, C_E1:C_E1H] = (
        ((np.arange(128) % HP) < HD).astype(np.float32)[:, None] / 256.0)
    for h2 in range(4):
        cstm[h2 * HP: h2 * HP + HD, C_E1H + h2] = 1.0 / 256.0
    for j in range(2):
        cstm[0:4, C_AL + j] = sim_alpha[j * 4:(j + 1) * 4]
    for h2 in range(4):
        for j in range(2):
            cstm[0:64, C_BB + h2 * 8 + j * 4: C_BB + h2 * 8 + (j + 1) * 4] = \
                sim_beta[j * 4:(j + 1) * 4][None, :]
    cstm[:, C_B2:C_END] = b2.reshape(3, 128).T

    R = _upsample_R()
    A = R.T  # (8 in, 64 out)
    upw = (A[:, None, :, None] * A[None, :, None, :]).reshape(64, 4096)
    upw2 = upw.astype(np.float16)         # exact: multiples of 1/256

    return dict(cst=cstm, upw2=upw2)


def _get_nc():
    if "nc" not in _CACHE:
        _CACHE["nc"] = _build_nc()
    return _CACHE["nc"]


def run(inputs, trace=False):
    nc = _get_nc()
    consts = _host_prep(inputs["W1"], inputs["b1"], inputs["W2"], inputs["b2"],
                        inputs["sim_alpha"], inputs["sim_beta"])
    x = np.ascontiguousarray(np.asarray(inputs["x"], np.float32))
    in_maps = []
    for i in range(NCORES):
        m = {"xc": np.ascontiguousarray(x[i * BPC:(i + 1) * BPC])}
        m.update(consts)
        in_maps.append(m)
    res = run_bass_kernel_spmd(nc, in_maps, core_ids=list(range(NCORES)),
                               trace=trace)
    out = np.concatenate(
        [res.results[i]["outc"] for i in range(NCORES)], axis=0
    ).astype(np.float32)
    return out, res


def kernel(**inputs):
    out, _ = run(inputs, trace=False)
    return out
